# revision 13
# baseline (speedup 1.0000x reference)
"""Trainium2 Bass kernel for a dense transformer encoder block.

Sharding: 8 cores = 4 batches x 2 query-halves, no collectives. Each core's
kv sequence is host-reordered so its 1024 query tokens come first (attention
is permutation-invariant over keys), so Q/residual tensors are plain slices
of the kv set. K/V are computed for the full 2048-token sequence (~6% extra
FLOPs vs. perfect sharding).

Per-core dataflow is feature-major ("T" = [feature, token]) so every matmul
has contraction dim 128 on partitions (sub-128 contraction fails to load on
this stack):
  LN1 (token-major, bn_stats) -> PE-transpose -> xnT           [phase 1]
  per head-group (4 heads): Q/K/V projections from xnT;        [phase 3]
    scores^T = KT_pair^T @ Qpad (zero-padded rhs selects one head)
    exp on ACT (scale=1/8 fused) -> f32r
    AV: two M=64 col-tiled matmuls (psum rows 0:64 / 64:128)
    den: two M=1 col-tiled ones-matmuls (psum rows 0 / 32)
    divide via DMA partition-broadcast reciprocal -> RT
  O-proj +bo, PE-transpose back, +residual -> x2 -> DRAM       [phase 4a]
  LN2 on x2 -> PE-transpose -> xn2T                            [phase 4b]
  MLP: h1 (+b1, exact Gelu on ACT), h2 (+b2), transpose back,  [phase 5]
    +x2 residual -> out

Matmuls run in float32r (TF32-class, ~1.5e-4 rel err, full PE rate at
free-dim >= 256); accumulation is fp32 in PSUM.
"""

import os
import sys

sys.path.insert(0, "/opt/trn_rl_repo")

from contextlib import ExitStack

import numpy as np

import concourse.bass as bass
import concourse.tile as tile
from concourse import bacc, bass_utils, mybir
from concourse.masks import make_identity

F32 = mybir.dt.float32
F32R = mybir.dt.float32r
AF = mybir.ActivationFunctionType
ALU = mybir.AluOpType

B, S, D = 4, 2048, 1024
H, DH, MLP = 16, 64, 4096
P = 128
KD = D // P            # 8 partition tiles over D
FT = MLP // P          # 32 partition tiles over MLP dim
NQ = S // 2            # 1024 query tokens per core
ST = S // P            # 16 kv token tiles
QTT = NQ // P          # 8 q token tiles
QS = 512               # free-dim slice
NQS = NQ // QS         # 2
NKS = S // QS          # 4
NG = 4                 # head groups
EPS = 1e-6
DEBUG = bool(int(os.environ.get("KERNEL_DEBUG", "0")))

_CACHE = {}


def _build():
    nc = bacc.Bacc(None, target_bir_lowering=False, debug=False, num_devices=8)

    xkv = nc.dram_tensor("xkv", [S, D], F32, kind="ExternalInput").ap()
    Wq = nc.dram_tensor("Wq", [D, D], F32R, kind="ExternalInput").ap()
    Wk = nc.dram_tensor("Wk", [D, D], F32R, kind="ExternalInput").ap()
    Wv = nc.dram_tensor("Wv", [D, D], F32R, kind="ExternalInput").ap()
    Wo = nc.dram_tensor("Wo", [D, D], F32R, kind="ExternalInput").ap()
    W1 = nc.dram_tensor("W1", [D, MLP], F32R, kind="ExternalInput").ap()
    W2 = nc.dram_tensor("W2", [MLP, D], F32R, kind="ExternalInput").ap()
    bq = nc.dram_tensor("bq", [D], F32, kind="ExternalInput").ap()
    bk = nc.dram_tensor("bk", [D], F32, kind="ExternalInput").ap()
    bv = nc.dram_tensor("bv", [D], F32, kind="ExternalInput").ap()
    bo = nc.dram_tensor("bo", [D], F32, kind="ExternalInput").ap()
    b1 = nc.dram_tensor("b1", [MLP], F32, kind="ExternalInput").ap()
    b2 = nc.dram_tensor("b2", [D], F32, kind="ExternalInput").ap()
    g1 = nc.dram_tensor("g1", [D], F32, kind="ExternalInput").ap()
    be1 = nc.dram_tensor("be1", [D], F32, kind="ExternalInput").ap()
    g2 = nc.dram_tensor("g2", [D], F32, kind="ExternalInput").ap()
    be2 = nc.dram_tensor("be2", [D], F32, kind="ExternalInput").ap()
    out = nc.dram_tensor("out", [NQ, D], F32, kind="ExternalOutput").ap()

    dbg = {}
    if DEBUG:
        dbg["xnkvT"] = nc.dram_tensor("d_xnkvT", [P, KD, S], F32R, kind="ExternalOutput").ap()
        dbg["qt0"] = nc.dram_tensor("d_qt0", [P, 2, NQ], F32R, kind="ExternalOutput").ap()
        dbg["kt0"] = nc.dram_tensor("d_kt0", [P, 2, S], F32R, kind="ExternalOutput").ap()
        dbg["v0"] = nc.dram_tensor("d_v0", [P, ST, 2, 2, P], F32R, kind="ExternalOutput").ap()
        dbg["rt"] = nc.dram_tensor("d_rt", [P, KD, NQ], F32R, kind="ExternalOutput").ap()
        dbg["e0"] = nc.dram_tensor("d_e0", [P, QS], F32R, kind="ExternalOutput").ap()
        dbg["s0"] = nc.dram_tensor("d_s0", [P, QS], F32, kind="ExternalOutput").ap()
        dbg["av0"] = nc.dram_tensor("d_av0", [P, QS], F32, kind="ExternalOutput").ap()
        dbg["den0"] = nc.dram_tensor("d_den0", [33, QS], F32, kind="ExternalOutput").ap()
        dbg["bc0"] = nc.dram_tensor("d_bc0", [P, QS], F32, kind="ExternalOutput").ap()
        dbg["x2"] = nc.dram_tensor("d_x2", [P, QTT, D], F32, kind="ExternalOutput").ap()

    def bcast_ap(vec):
        # [D] dram vector -> [128, D] partition-replicated DMA source
        return bass.AP(tensor=vec.tensor, offset=vec.offset, ap=[[0, P]] + list(vec.ap))

    def wslice(W, c0, n):
        # weight col-slice -> [128, K/128, n] (contraction inner on partitions)
        return W[:, c0:c0 + n].rearrange("(ko p) m -> p ko m", p=P)

    with tile.TileContext(nc) as tc:
        es = ExitStack()
        params = es.enter_context(tc.tile_pool(name="params", bufs=1))
        dramp = es.enter_context(tc.tile_pool(name="dram", bufs=1, space="DRAM"))
        x2d = dramp.tile([P, QTT, D], F32)

        ident_f = params.tile([P, P], F32)
        make_identity(nc, ident_f)
        ident = params.tile([P, P], F32R)
        nc.vector.tensor_copy(ident[:], ident_f[:])
        ones_f = params.tile([P, 33], F32)
        nc.vector.memset(ones_f[:], 0.0)
        nc.vector.memset(ones_f[:, 0:1], 1.0)
        onesA_r = params.tile([P, 33], F32R)
        nc.vector.tensor_copy(onesA_r[:], ones_f[:])
        nc.vector.memset(ones_f[:, 0:1], 0.0)
        nc.vector.memset(ones_f[:, 32:33], 1.0)
        onesB_r = params.tile([P, 33], F32R)
        nc.vector.tensor_copy(onesB_r[:], ones_f[:])

        def pvec(v, n, nm):  # [n*128] -> [128, n] (dim o*128+p -> [p, o])
            t = params.tile([P, n], F32, name=nm)
            nc.sync.dma_start(t[:], v.rearrange("(o p) -> p o", p=P))
            return t

        bq_t = pvec(bq, KD, "bq_t")
        bk_t = pvec(bk, KD, "bk_t")
        bo_t = pvec(bo, KD, "bo_t")
        b2_t = pvec(b2, KD, "b2_t")
        b1_t = pvec(b1, FT, "b1_t")
        bv_rep = params.tile([P, D], F32)
        nc.gpsimd.dma_start(bv_rep[:], bcast_ap(bv))

        rt_es = ExitStack()
        rtp = rt_es.enter_context(tc.tile_pool(name="rt", bufs=1))
        RT = rtp.tile([P, KD, NQ], F32R)

        xn_es = ExitStack()
        xnp = xn_es.enter_context(tc.tile_pool(name="xn", bufs=1))
        xn_kvT = xnp.tile([P, KD, S], F32R)

        # ---- Phase 1: LN1 + transpose to feature-major ----
        with tc.tile_pool(name="p1tmp", bufs=3) as p1t, \
             tc.tile_pool(name="p1s", bufs=4) as p1s, \
             tc.tile_pool(name="ln1", bufs=1) as ln1p, \
             tc.tile_pool(name="p1ps", bufs=4, space="PSUM") as ps1:
            g1_rep = ln1p.tile([P, D], F32)
            nc.gpsimd.dma_start(g1_rep[:], bcast_ap(g1))
            be1_rep = ln1p.tile([P, D], F32)
            nc.gpsimd.dma_start(be1_rep[:], bcast_ap(be1))
            eps_t = ln1p.tile([P, 1], F32)
            nc.vector.memset(eps_t[:], EPS)

            for t in range(ST):
                x_t = p1t.tile([P, D], F32, tag="x_t")
                nc.sync.dma_start(x_t[:], xkv[t * P:(t + 1) * P, :])
                stats = p1s.tile([P, 2, 6], F32, tag="stats")
                xv = x_t[:].rearrange("p (s f) -> p s f", s=2)
                for s in range(2):
                    nc.vector.bn_stats(stats[:, s, :], xv[:, s, :])
                mv = p1s.tile([P, 2], F32, tag="mv")
                nc.vector.bn_aggr(mv[:], stats[:])
                std = p1s.tile([P, 1], F32, tag="std")
                nc.scalar.activation(std[:], mv[:, 1:2], AF.Sqrt, bias=eps_t[:])
                nc.vector.reciprocal(std[:], std[:])
                xn_t = p1t.tile([P, D], F32R, tag="xn_t")
                nc.vector.tensor_scalar(
                    xn_t[:], x_t[:], scalar1=mv[:, 0:1], scalar2=std[:],
                    op0=ALU.subtract, op1=ALU.mult)
                nc.vector.tensor_tensor(xn_t[:], xn_t[:], g1_rep[:], ALU.mult)
                nc.vector.tensor_tensor(xn_t[:], xn_t[:], be1_rep[:], ALU.add)
                for j in range(KD):
                    pst = ps1.tile([P, P], F32, tag="tp")
                    nc.tensor.transpose(pst[:].bitcast(F32R), xn_t[:, j * P:(j + 1) * P], ident[:])
                    nc.vector.tensor_copy(xn_kvT[:, j, t * P:(t + 1) * P], pst[:])

        if DEBUG:
            nc.sync.dma_start(dbg["xnkvT"], xn_kvT[:])

        # ---- Phase 3: per-group QKV projection + attention ----
        with tc.tile_pool(name="kv", bufs=1) as kvp, \
             tc.tile_pool(name="wst", bufs=2) as wsp, \
             tc.tile_pool(name="expp", bufs=2) as expp, \
             tc.tile_pool(name="qpad", bufs=1) as qpp, \
             tc.tile_pool(name="rcbc", bufs=1) as rcp, \
             tc.tile_pool(name="aps", bufs=1, space="PSUM") as aps:

            zsc = qpp.tile([P, QS], F32)
            nc.vector.memset(zsc[:], 0.0)
            qpadA = [qpp.tile([P, QS], F32R, name=f"qpadA{i}") for i in range(2)]
            qpadB = [qpp.tile([P, QS], F32R, name=f"qpadB{i}") for i in range(2)]
            for i in range(2):
                nc.vector.tensor_copy(qpadA[i][:], zsc[:])
                nc.vector.tensor_copy(qpadB[i][:], zsc[:])

            QT_g = kvp.tile([P, 2, NQ], F32R)
            KT_g = kvp.tile([P, 2, S], F32R)
            V_gp = kvp.tile([P, ST, 2, 2, P], F32R)
            for t in range(ST):
                nc.vector.tensor_copy(
                    V_gp[:, t], zsc[:].rearrange("p (a b m) -> p a b m", a=2, b=2))
            it_count = 0

            for g in range(NG):
                for pl in range(2):   # head pairs 2g, 2g+1
                    pr = 2 * g + pl
                    wq_t = wsp.tile([P, KD, P], F32R, tag="wq_t")
                    nc.sync.dma_start(wq_t[:], wslice(Wq, pr * P, P))
                    for q in range(NQS):
                        ps = aps.tile([P, QS], F32, tag="pp", bufs=2)
                        for kd in range(KD):
                            nc.tensor.matmul(
                                ps[:], wq_t[:, kd, :], xn_kvT[:, kd, q * QS:(q + 1) * QS],
                                start=(kd == 0), stop=(kd == KD - 1))
                        nc.vector.tensor_scalar_add(
                            QT_g[:, pl, q * QS:(q + 1) * QS], ps[:], bq_t[:, pr:pr + 1])
                    wk_t = wsp.tile([P, KD, P], F32R, tag="wk_t")
                    nc.sync.dma_start(wk_t[:], wslice(Wk, pr * P, P))
                    for q in range(NKS):
                        ps = aps.tile([P, QS], F32, tag="pp", bufs=2)
                        for kd in range(KD):
                            nc.tensor.matmul(
                                ps[:], wk_t[:, kd, :], xn_kvT[:, kd, q * QS:(q + 1) * QS],
                                start=(kd == 0), stop=(kd == KD - 1))
                        nc.vector.tensor_scalar_add(
                            KT_g[:, pl, q * QS:(q + 1) * QS], ps[:], bk_t[:, pr:pr + 1])
                wv_t = wsp.tile([P, KD, 256], F32R, tag="wv_t", bufs=1)
                nc.sync.dma_start(wv_t[:], wslice(Wv, g * 256, 256))
                for t in range(ST):
                    ps = aps.tile([P, QS], F32, tag="pp", bufs=2)
                    for kd in range(KD):
                        nc.tensor.matmul(
                            ps[:, 0:256], xn_kvT[:, kd, t * P:(t + 1) * P], wv_t[:, kd, :],
                            start=(kd == 0), stop=(kd == KD - 1))
                    for pi in range(2):
                        nc.vector.tensor_tensor(
                            V_gp[:, t, pi, 0, 0:64], ps[:, pi * 128:pi * 128 + 64],
                            bv_rep[:, g * 256 + pi * 128:g * 256 + pi * 128 + 64], ALU.add)
                        nc.vector.tensor_tensor(
                            V_gp[:, t, pi, 1, 64:128], ps[:, pi * 128 + 64:pi * 128 + 128],
                            bv_rep[:, g * 256 + pi * 128 + 64:g * 256 + pi * 128 + 128], ALU.add)

                if DEBUG and g == 0:
                    nc.sync.dma_start(dbg["kt0"], KT_g[:])
                    nc.sync.dma_start(dbg["v0"], V_gp[:])
                    nc.sync.dma_start(dbg["qt0"], QT_g[:])

                for q in range(NQS):
                    for pl in range(2):
                        pr = 2 * g + pl
                        i = it_count % 2
                        it_count += 1
                        qsl = slice(q * QS, (q + 1) * QS)
                        nc.vector.tensor_copy(qpadA[i][0:64, :], QT_g[0:64, pl, qsl])
                        nc.vector.tensor_copy(qpadB[i][64:128, :], QT_g[64:128, pl, qsl])
                        av = aps.tile([P, QS], F32, tag="av")
                        den = aps.tile([P, QS], F32, tag="den")
                        for kt in range(ST):
                            ktsl = slice(kt * P, (kt + 1) * P)
                            sA = aps.tile([P, QS], F32, tag="sA", bufs=2)
                            sB = aps.tile([P, QS], F32, tag="sB", bufs=2)
                            nc.tensor.matmul(sA[:], KT_g[:, pl, ktsl], qpadA[i][:],
                                             start=True, stop=True)
                            nc.tensor.matmul(sB[:], KT_g[:, pl, ktsl], qpadB[i][:],
                                             start=True, stop=True)
                            eA = expp.tile([P, QS], F32R, tag="eA")
                            eB = expp.tile([P, QS], F32R, tag="eB")
                            nc.scalar.activation(eA[:], sA[:], AF.Exp, scale=0.125)
                            nc.scalar.activation(eB[:], sB[:], AF.Exp, scale=0.125)
                            if DEBUG and g == 0 and q == 0 and pl == 0 and kt == 0:
                                nc.sync.dma_start(dbg["e0"], eA[:])
                                s0c = rcp.tile([P, QS], F32, tag="s0c")
                                nc.vector.tensor_copy(s0c[:], sA[:])
                                nc.sync.dma_start(dbg["s0"], s0c[:])
                            st, sp = (kt == 0), (kt == ST - 1)
                            nc.tensor.matmul(av[:], V_gp[:, kt, pl, 0, :], eA[:],
                                             start=st, stop=False, skip_group_check=True)
                            nc.tensor.matmul(av[:], V_gp[:, kt, pl, 1, :], eB[:],
                                             start=False, stop=sp, skip_group_check=True)
                            nc.tensor.matmul(den[0:33, :], onesA_r[:], eA[:],
                                             start=st, stop=False, skip_group_check=True)
                            nc.tensor.matmul(den[0:33, :], onesB_r[:], eB[:],
                                             start=False, stop=sp, skip_group_check=True)
                        rc = rcp.tile([33, QS], F32, tag="rc")
                        nc.vector.tensor_copy(rc[:], den[0:33, :])
                        if DEBUG and g == 0 and q == 0 and pl == 0:
                            nc.sync.dma_start(dbg["den0"], rc[:])
                            av0c = rcp.tile([P, QS], F32, tag="av0c")
                            nc.vector.tensor_copy(av0c[:], av[:])
                            nc.sync.dma_start(dbg["av0"], av0c[:])
                        nc.vector.reciprocal(rc[0:1, :], rc[0:1, :])
                        nc.vector.reciprocal(rc[32:33, :], rc[32:33, :])
                        rcd = dramp.tile([2, QS], F32, tag="rcd", bufs=2)
                        nc.sync.dma_start(rcd[0:1, :], rc[0:1, :])
                        nc.sync.dma_start(rcd[1:2, :], rc[32:33, :])
                        bc = rcp.tile([P, QS], F32, tag="bc")

                        def _b64(row_ap):
                            return bass.AP(tensor=row_ap.tensor, offset=row_ap.offset,
                                           ap=[[0, 64]] + list(row_ap.ap)[1:])

                        nc.sync.dma_start(bc[0:64, :], _b64(rcd[0:1, :]))
                        nc.sync.dma_start(bc[64:128, :], _b64(rcd[1:2, :]))
                        if DEBUG and g == 0 and q == 0 and pl == 0:
                            nc.sync.dma_start(dbg["bc0"], bc[:])
                        nc.vector.tensor_tensor(RT[:, pr, qsl], av[:], bc[:], ALU.mult)

        xn_es.close()

        if DEBUG:
            nc.sync.dma_start(dbg["rt"], RT[:])

        # ---- Phase 4a: O-projection + residual -> x2 (DRAM) ----
        with tc.tile_pool(name="p4tmp", bufs=2) as p4t, \
             tc.tile_pool(name="p4ps", bufs=2, space="PSUM") as ps4, \
             tc.tile_pool(name="p4tps", bufs=4, space="PSUM") as ps4t:
            for q in range(NQS):
                attnT = p4t.tile([P, KD, QS], F32R, tag="attnT")
                for mt in range(KD):
                    wo_t = p4t.tile([P, KD, P], F32R, tag="wo_t")
                    nc.sync.dma_start(wo_t[:], wslice(Wo, mt * P, P))
                    ps = ps4.tile([P, QS], F32, tag="pp")
                    for kd in range(KD):
                        nc.tensor.matmul(
                            ps[:], wo_t[:, kd, :], RT[:, kd, q * QS:(q + 1) * QS],
                            start=(kd == 0), stop=(kd == KD - 1))
                    nc.vector.tensor_scalar_add(
                        attnT[:, mt, :], ps[:], bo_t[:, mt:mt + 1])
                for j in range(QS // P):
                    tt = q * (QS // P) + j
                    xr_t = p4t.tile([P, D], F32, tag="xr_t")
                    nc.sync.dma_start(xr_t[:], xkv[tt * P:(tt + 1) * P, :])
                    x2_t = p4t.tile([P, D], F32, tag="x2_t")
                    for mt in range(KD):
                        pst = ps4t.tile([P, P], F32, tag="tp")
                        nc.tensor.transpose(pst[:].bitcast(F32R),
                                            attnT[:, mt, j * P:(j + 1) * P], ident[:])
                        nc.vector.tensor_tensor(
                            x2_t[:, mt * P:(mt + 1) * P], pst[:],
                            xr_t[:, mt * P:(mt + 1) * P], ALU.add)
                    nc.sync.dma_start(x2d[:, tt, :], x2_t[:])
                    if DEBUG:
                        nc.sync.dma_start(dbg["x2"][:, tt, :], x2_t[:])
        rt_es.close()

        # ---- Phase 4b: LN2 -> xn2T ----
        xn2_es = ExitStack()
        xn2p = xn2_es.enter_context(tc.tile_pool(name="xn2", bufs=1))
        xn2T = xn2p.tile([P, KD, NQ], F32R)
        with tc.tile_pool(name="p4btmp", bufs=3) as p4bt, \
             tc.tile_pool(name="p4bs", bufs=4) as p4bs, \
             tc.tile_pool(name="ln2", bufs=1) as ln2p, \
             tc.tile_pool(name="p4bps", bufs=4, space="PSUM") as ps4b:
            g2_rep = ln2p.tile([P, D], F32)
            nc.gpsimd.dma_start(g2_rep[:], bcast_ap(g2))
            be2_rep = ln2p.tile([P, D], F32)
            nc.gpsimd.dma_start(be2_rep[:], bcast_ap(be2))
            eps2_t = ln2p.tile([P, 1], F32)
            nc.vector.memset(eps2_t[:], EPS)

            for tt in range(QTT):
                x2_t = p4bt.tile([P, D], F32, tag="x2_t")
                nc.sync.dma_start(x2_t[:], x2d[:, tt, :])
                stats = p4bs.tile([P, 2, 6], F32, tag="stats2")
                xv = x2_t[:].rearrange("p (s f) -> p s f", s=2)
                for s in range(2):
                    nc.vector.bn_stats(stats[:, s, :], xv[:, s, :])
                mv = p4bs.tile([P, 2], F32, tag="mv2")
                nc.vector.bn_aggr(mv[:], stats[:])
                std = p4bs.tile([P, 1], F32, tag="std2")
                nc.scalar.activation(std[:], mv[:, 1:2], AF.Sqrt, bias=eps2_t[:])
                nc.vector.reciprocal(std[:], std[:])
                xn2_t = p4bt.tile([P, D], F32R, tag="xn2_t")
                nc.vector.tensor_scalar(
                    xn2_t[:], x2_t[:], scalar1=mv[:, 0:1], scalar2=std[:],
                    op0=ALU.subtract, op1=ALU.mult)
                nc.vector.tensor_tensor(xn2_t[:], xn2_t[:], g2_rep[:], ALU.mult)
                nc.vector.tensor_tensor(xn2_t[:], xn2_t[:], be2_rep[:], ALU.add)
                for j in range(KD):
                    pst = ps4b.tile([P, P], F32, tag="tp")
                    nc.tensor.transpose(pst[:].bitcast(F32R), xn2_t[:, j * P:(j + 1) * P], ident[:])
                    nc.vector.tensor_copy(xn2T[:, j, tt * P:(tt + 1) * P], pst[:])

        # ---- Phase 5: MLP ----
        with tc.tile_pool(name="p5tmp", bufs=3) as p5t, \
             tc.tile_pool(name="h1", bufs=1) as h1p, \
             tc.tile_pool(name="w2st", bufs=2) as w2p, \
             tc.tile_pool(name="p5ps", bufs=2, space="PSUM") as ps5, \
             tc.tile_pool(name="p5tps", bufs=4, space="PSUM") as ps5t:
            for sl in range(NQS):
                ssl = slice(sl * QS, (sl + 1) * QS)
                h1T = h1p.tile([P, FT, QS], F32R, tag="h1T")
                for ft in range(FT):
                    w1_t = p5t.tile([P, KD, P], F32R, tag="w1_t")
                    nc.sync.dma_start(w1_t[:], wslice(W1, ft * P, P))
                    ps = ps5.tile([P, QS], F32, tag="pp")
                    for kd in range(KD):
                        nc.tensor.matmul(
                            ps[:], w1_t[:, kd, :], xn2T[:, kd, ssl],
                            start=(kd == 0), stop=(kd == KD - 1))
                    nc.scalar.activation(h1T[:, ft, :], ps[:], AF.Gelu,
                                         bias=b1_t[:, ft:ft + 1])
                outT = p5t.tile([P, KD, QS], F32R, tag="outT", bufs=2)
                for mt in range(KD):
                    w2_t = w2p.tile([P, FT, P], F32R, tag="w2_t")
                    nc.sync.dma_start(w2_t[:], wslice(W2, mt * P, P))
                    ps = ps5.tile([P, QS], F32, tag="pp")
                    for ft in range(FT):
                        nc.tensor.matmul(
                            ps[:], w2_t[:, ft, :], h1T[:, ft, :],
                            start=(ft == 0), stop=(ft == FT - 1))
                    nc.vector.tensor_scalar_add(
                        outT[:, mt, :], ps[:], b2_t[:, mt:mt + 1])
                for j in range(QS // P):
                    tt = sl * (QS // P) + j
                    x2_t = p5t.tile([P, D], F32, tag="x2r_t")
                    nc.sync.dma_start(x2_t[:], x2d[:, tt, :])
                    out_sb = p5t.tile([P, D], F32, tag="out_sb")
                    for mt in range(KD):
                        pst = ps5t.tile([P, P], F32, tag="tp")
                        nc.tensor.transpose(pst[:].bitcast(F32R),
                                            outT[:, mt, j * P:(j + 1) * P], ident[:])
                        nc.vector.tensor_tensor(
                            out_sb[:, mt * P:(mt + 1) * P], pst[:],
                            x2_t[:, mt * P:(mt + 1) * P], ALU.add)
                    nc.sync.dma_start(out[tt * P:(tt + 1) * P, :], out_sb[:])

        xn2_es.close()
        es.close()

    nc.compile()
    return nc


def kernel(**inputs):
    inputs = {k: np.ascontiguousarray(np.asarray(v), dtype=np.float32)
              for k, v in inputs.items()}
    if "nc" not in _CACHE:
        _CACHE["nc"] = _build()
    nc = _CACHE["nc"]

    x = inputs["x"]
    shared = {
        "Wq": inputs["Wq"], "Wk": inputs["Wk"], "Wv": inputs["Wv"], "Wo": inputs["Wo"],
        "W1": inputs["W1"], "W2": inputs["W2"],
        "bq": inputs["bq"], "bk": inputs["bk"], "bv": inputs["bv"], "bo": inputs["bo"],
        "b1": inputs["b1"], "b2": inputs["b2"],
        "g1": inputs["ln1_g"], "be1": inputs["ln1_b"],
        "g2": inputs["ln2_g"], "be2": inputs["ln2_b"],
    }
    in_maps = []
    for c in range(8):
        b, half = c // 2, c % 2
        m = dict(shared)
        # query half first; attention is permutation-invariant over kv order
        m["xkv"] = np.ascontiguousarray(
            np.concatenate([x[b, half * NQ:(half + 1) * NQ, :],
                            x[b, (1 - half) * NQ:(2 - half) * NQ, :]], axis=0))
        in_maps.append(m)

    res = bass_utils.run_bass_kernel_spmd(nc, in_maps, core_ids=list(range(8)))
    _CACHE["last_results"] = res

    outa = np.empty((B, S, D), dtype=np.float32)
    for c in range(8):
        b, half = c // 2, c % 2
        outa[b, half * NQ:(half + 1) * NQ, :] = res.results[c]["out"]
    return outa
